# revision 37
# baseline (speedup 1.0000x reference)
"""Causal self-attention kernel for 8 trn2 NeuronCores.

Sharding: core c handles batch b = c // 4 and local head group hg = c % 4
(4 of the 16 heads). Tensor-parallel over heads for kqv / attention and
row-parallel for the output projection; the 4 per-batch partial projections
are summed on the host (the "all-reduce" of classic TP), where the bias is
also added.

Inputs are pre-tiled on the host into [128, *] SBUF-ready layouts (128-row
panels packed along the free dim) so every load is a single large DMA.

Device kernel (per core, bf16 matmuls, fp32 accumulation), fully software-
pipelined with a simulated-time greedy scheduler at trace time:

  score strip p = two tk blocks (2m, 2m+1) of one (window g, head pair hp):
      4 S^T matmuls (the two heads of the pair run concurrently on PE row
      groups 0-63 / 64-127) into one [128, 2048] PSUM strip, then ONE
      ScalarE exp (amortizes the ~350-cycle ACTIVATE overhead), then a
      VectorE multiply with a precomputed triangular bf16 mask for the
      diagonal squares.  pt strips park in an SBUF ring ahead of use.
  pv(g,hp,j):  O^T psum[0:64] += v_j.T @ P ; psum[64:128] += ones.T @ P
  normalize:   l rows -> SBUF (GpSimd, off both hot engines), VectorE
               reciprocal_approx_fast, VectorE multiply into oc
  dense units: kq / v / proj matmuls + PSUM->SBUF copies, interleaved as
               filler wherever the PE would otherwise wait on exp

The scheduler keeps the PE continuously busy (which also keeps the HAM
clock-gate at 8/8); a burst of warm-up matmuls on scratch data warms the
clock before the first real matmul, and the input DMA is split finely so
the first kq matmuls start as soon as their slices land.
"""

import numpy as np
import ml_dtypes

T = 2048
C = 1024
NH_LOCAL = 4
D = 64
TQW = 512  # tq window width
NGRP = T // TQW  # 4 tq windows
RING_K = 28  # pt strip ring depth (one strip per tk block)

_nc_cache = {}


def _build_bass():
    import concourse.mybir as mybir
    import concourse.tile as tile
    from concourse import bacc

    f32 = mybir.dt.float32
    bf16 = mybir.dt.bfloat16

    nc = bacc.Bacc(None, target_bir_lowering=False)
    # pre-tiled inputs: [128, packed free dim] (see _shard_inputs)
    xt_d = nc.dram_tensor("xt", [128, 8 * T], bf16, kind="ExternalInput")
    wqk_d = nc.dram_tensor("wqk", [128, 8 * 512], bf16, kind="ExternalInput")
    wv_d = nc.dram_tensor("wv", [128, 8 * 256], bf16, kind="ExternalInput")
    wp_d = nc.dram_tensor("wp", [128, 2 * C], bf16, kind="ExternalInput")
    y_d = nc.dram_tensor("y", [T, C], f32, kind="ExternalOutput")

    with tile.TileContext(nc) as tc:
        with (
            tc.tile_pool(name="persist", bufs=1) as pp,
            tc.tile_pool(name="mmp", bufs=2, space="PSUM") as mp,
            tc.tile_pool(name="spsum", bufs=2, space="PSUM") as sp,
            tc.tile_pool(name="opsum", bufs=1, space="PSUM") as op,
            tc.tile_pool(name="ptp", bufs=RING_K) as ptp,
            tc.tile_pool(name="rp", bufs=2) as rp,
            tc.tile_pool(name="ysb", bufs=2) as ysb,
        ):
            xt_s = pp.tile([128, 8 * T], bf16, tag="xt", name="xt")
            wqk_s = pp.tile([128, 8 * 512], bf16, tag="wqk", name="wqk")
            wv_s = pp.tile([128, 8 * 256], bf16, tag="wv", name="wv")
            wp_s = pp.tile([128, 2 * C], bf16, tag="wp", name="wp")
            kq_s = [pp.tile([128, T], bf16, tag=f"kq{f}", name=f"kq{f}") for f in range(4)]
            v_s = pp.tile([128, 4 * T], bf16, tag="vall", name="vall")
            oc_s = [pp.tile([128, T], bf16, tag=f"oc{p}", name=f"oc{p}") for p in range(2)]
            scr_s = pp.tile([128, 512], bf16, tag="scr", name="scr")
            msk_s = pp.tile([128, 256], bf16, tag="msk", name="msk")

            # slicing helpers for the packed layouts
            def xt_w(c, g):  # moving operand for window g, contraction chunk c
                o = 4096 * g + 512 * c
                return xt_s[:, o : o + 512]

            def xt_j(c, j):  # stationary operand for v: tk chunk j
                o = 4096 * (j // 4) + 512 * c + 128 * (j % 4)
                return xt_s[:, o : o + 128]

            # ---- DMA: interleave wqk-f0 chunks with xt window-0 chunks so
            # the first kq matmuls start as soon as their slices land; the
            # rest ordered by first use.
            for c in range(8):
                nc.sync.dma_start(
                    wqk_s[:, 128 * c : 128 * (c + 1)], wqk_d[:, 128 * c : 128 * (c + 1)]
                )
                nc.sync.dma_start(
                    xt_s[:, 512 * c : 512 * (c + 1)], xt_d[:, 512 * c : 512 * (c + 1)]
                )
                if c == 1:
                    nc.sync.dma_start(wqk_s[:, 2048:3072], wqk_d[:, 2048:3072])  # q01
                if c == 3:
                    nc.sync.dma_start(wv_s[:], wv_d[:])
            nc.sync.dma_start(wqk_s[:, 1024:2048], wqk_d[:, 1024:2048])  # k23
            nc.sync.dma_start(wqk_s[:, 3072:4096], wqk_d[:, 3072:4096])  # q23
            for g in range(1, NGRP):
                nc.sync.dma_start(
                    xt_s[:, 4096 * g : 4096 * (g + 1)],
                    xt_d[:, 4096 * g : 4096 * (g + 1)],
                )
            nc.sync.dma_start(wp_s[:], wp_d[:])

            # ---- early engine setup (off the critical path) ----
            # causal mask (keep tq >= tk), duplicated for the two heads of a
            # pair so one VectorE multiply covers both diagonal squares
            nc.gpsimd.memset(msk_s[:], 1.0)
            for half in range(2):
                nc.gpsimd.affine_select(
                    out=msk_s[:, 128 * half : 128 * (half + 1)],
                    in_=msk_s[:, 128 * half : 128 * (half + 1)],
                    compare_op=mybir.AluOpType.is_ge,
                    fill=0.0,
                    base=0,
                    pattern=[[1, 128]],
                    channel_multiplier=-1,
                )
            nc.vector.memset(scr_s[:], 0.125)
            # ones blocks for the denominator trick (v halves overwritten
            # later); quarter 0 on VectorE (needed first), rest on GpSimd
            nc.vector.memset(v_s[:, 0:2048], 1.0)
            for q in range(1, 4):
                nc.gpsimd.memset(v_s[:, 2048 * q : 2048 * (q + 1)], 1.0)

            # =============== cost model (measured on HW) ===============
            RATE = 0.426  # ns per PE moving column (warm clock)
            VC_KQ = 830.0  # VectorE copy psum->sbuf, 512 fp32 cols
            VC_V = 510.0
            VC_PROJ = 830.0
            VC_MASK = 460.0
            VC_RECIP = 830.0
            VC_MULT = 830.0
            EXPC = lambda cols: cols * 0.84 + 300.0

            st8 = {
                "pe": 6300.0, "sc": 0.0, "vec": 6500.0, "gp": 8000.0,
                "stall": 0.0,
            }
            # rough DMA landing estimates (ns), matching emission order above
            DMA0, KBNS = 9000.0, 5.5  # first byte, ns per KB (measured ramp)
            cum = 0.0
            xt_avail = [0.0] * 8  # per window-0 chunk, then per window
            wqk_avail = [0.0] * 4
            for c in range(8):
                cum += 32
                wqk_avail[0] = DMA0 + cum * KBNS  # f0 chunk c (last wins)
                cum += 128
                xt_avail[c] = DMA0 + cum * KBNS
                if c == 1:
                    cum += 256; wqk_avail[2] = DMA0 + cum * KBNS
                if c == 3:
                    cum += 512; wv_avail = DMA0 + cum * KBNS
            cum += 256; wqk_avail[1] = DMA0 + cum * KBNS
            cum += 256; wqk_avail[3] = DMA0 + cum * KBNS
            xtw_avail = [max(xt_avail)]
            for g in range(1, NGRP):
                cum += 1024
                xtw_avail.append(DMA0 + cum * KBNS)
            cum += 512; wp_avail = DMA0 + cum * KBNS

            # =============== streams ===============
            # blocks: flat list over (g, hp, j); strips pair consecutive
            # blocks of the same (g, hp)
            blocks = []
            for g in range(NGRP):
                for hp in range(2):
                    for j in range(4 * g + 4):
                        cs = max(0, 128 * j - 512 * g)
                        blocks.append(
                            dict(g=g, hp=hp, j=j, cs=cs, w0=512 * g,
                                 diag=128 * j >= 512 * g, jmax=4 * g + 3)
                        )
            NB = len(blocks)
            NS = NB // 2  # strips

            # dense units ------------------------------------------------
            # each: dict(fns=[(pe_cost, fn)...], vend=None, key)
            def kq_unit(w, f):
                u = {"vend": 0.0, "key": f"kq{w}f{f}"}
                u["avail"] = max(wqk_avail[f], xtw_avail[w] if w else 0.0)
                st = {}
                fns = []
                for c in range(8):
                    av = max(
                        wqk_avail[f], xt_avail[c] if w == 0 else xtw_avail[w]
                    )
                    def mm(f=f, c=c, w=w, st=st, av=av):
                        if c == 0:
                            st["acc"] = mp.tile([128, 512], f32, tag="mm", name="mmkq")
                        st8["pe"] = max(st8["pe"], av)
                        nc.tensor.matmul(
                            st["acc"][:],
                            wqk_s[:, 1024 * f + 128 * c : 1024 * f + 128 * (c + 1)],
                            xt_w(c, w),
                            start=(c == 0),
                            stop=(c == 7),
                        )
                    fns.append((512 * RATE + 15, mm))
                def cp(f=f, w=w, st=st, u=u):
                    nc.vector.tensor_copy(
                        kq_s[f][:, TQW * w : TQW * (w + 1)], st["acc"][:]
                    )
                    st8["vec"] = max(st8["vec"], st8["pe"]) + VC_KQ
                    u["vend"] = st8["vec"]
                fns.append((0.0, cp))
                u["fns"] = fns
                return u

            def v_unit(j):
                u = {"vend": 0.0, "key": f"v{j}"}
                u["avail"] = max(wv_avail, xtw_avail[j // 4])
                st = {}
                fns = []
                for c in range(8):
                    def mm(j=j, c=c, st=st, u=u):
                        if c == 0:
                            st["acc"] = mp.tile([128, 512], f32, tag="mm", name="mmv")
                            st8["pe"] = max(st8["pe"], u["avail"])
                        nc.tensor.matmul(
                            st["acc"][:, :256],
                            xt_j(c, j),
                            wv_s[:, 256 * c : 256 * (c + 1)],
                            start=(c == 0),
                            stop=(c == 7),
                        )
                    fns.append((256 * RATE + 15, mm))
                def cp(j=j, st=st, u=u):
                    nc.vector.tensor_copy(
                        v_s[:, 512 * j : 512 * j + 512].rearrange(
                            "p (h x) -> p h x", h=4
                        )[:, :, 0:64],
                        st["acc"][:, 0:256].rearrange("p (h x) -> p h x", h=4),
                    )
                    st8["vec"] = max(st8["vec"], st8["pe"]) + VC_V
                    u["vend"] = st8["vec"]
                fns.append((0.0, cp))
                u["fns"] = fns
                return u

            wnorm_end = [0.0] * NGRP  # when window g's oc is fully written

            def proj_units(g):
                units = []
                for i in range(4 * g, 4 * g + 4):
                    u = {"vend": 0.0, "key": f"proj{i}", "avail": wp_avail}
                    st = {}
                    fns = []
                    def alloc(st=st):
                        st["ys"] = ysb.tile([128, C], f32, tag="ys", name="ys")
                    fns.append((0.0, alloc))
                    for uu in range(2):
                        for ci in range(2):
                            def mm(i=i, uu=uu, ci=ci, st=st, g=g):
                                if ci == 0:
                                    st["acc"] = mp.tile([128, 512], f32, tag="mm", name="mmy")
                                    st8["pe"] = max(st8["pe"], wnorm_end[g])
                                nc.tensor.matmul(
                                    st["acc"][:],
                                    oc_s[ci][:, 128 * i : 128 * (i + 1)],
                                    wp_s[:, 1024 * ci + 512 * uu : 1024 * ci + 512 * (uu + 1)],
                                    start=(ci == 0),
                                    stop=(ci == 1),
                                )
                            fns.append((512 * RATE + 15, mm))
                        def cpout(i=i, uu=uu, st=st):
                            if st8["sc"] + 1500.0 < st8["pe"]:
                                # ScalarE idle (post-exp tail): copy there
                                nc.scalar.copy(
                                    st["ys"][:, 512 * uu : 512 * (uu + 1)], st["acc"][:]
                                )
                                st8["sc"] = max(st8["sc"], st8["pe"]) + 700.0
                            else:
                                nc.vector.tensor_copy(
                                    st["ys"][:, 512 * uu : 512 * (uu + 1)], st["acc"][:]
                                )
                                st8["vec"] = max(st8["vec"], st8["pe"]) + VC_PROJ
                            nc.sync.dma_start(
                                y_d[128 * i : 128 * (i + 1), 512 * uu : 512 * (uu + 1)],
                                st["ys"][:, 512 * uu : 512 * (uu + 1)],
                            )
                        fns.append((0.0, cpout))
                    u["fns"] = fns
                    units.append(u)
                return units

            # deadlines: block index of first consumer --------------------
            def first_block(pred):
                for bi, b in enumerate(blocks):
                    if pred(b):
                        return bi
                return NB

            dense = []  # entries [deadline, kind, unit]
            for w in range(NGRP):
                for f in range(4):
                    if w == 0 and f in (0, 2):
                        dl = 0
                    elif f in (0, 1):
                        dl = first_block(lambda b, w=w, f=f: b["hp"] == f and b["j"] // 4 == w)
                    else:
                        dl = first_block(lambda b, w=w, f=f: b["g"] == w and b["hp"] == f - 2)
                    dense.append([dl, "kq", kq_unit(w, f)])
            for j in range(4 * NGRP):
                dl = first_block(lambda b, j=j: b["g"] == j // 4 and b["hp"] == 0 and b["j"] == j)
                dense.append([dl, "v", v_unit(j)])
            dense.sort(key=lambda e: e[0])

            kq_units_by = {}
            for dl, kind, u in dense:
                kq_units_by[u["key"]] = u

            # =============== emission ===============
            exp_end = [0.0] * NB
            mask_end = [0.0] * NB
            strip_pt = [None] * NB
            o_t = [None]
            norm_end = [0.0]
            ndummy = [0]

            def emit_dummy():
                wm = mp.tile([128, 512], f32, tag="mm", name="warm")
                nc.tensor.matmul(
                    wm[:], scr_s[:, 0:128], scr_s[:], start=True, stop=True
                )
                st8["pe"] += 300.0
                ndummy[0] += 1

            def run_unit(u):
                u["emitted"] = True
                for cost, fn in u["fns"]:
                    fn()
                    st8["pe"] += cost

            def score_avail(bi):
                b = blocks[bi]
                a = 0.0
                for key in (
                    f"kq{b['g']}f{2 + b['hp']}",
                    f"kq{b['j'] // 4}f{b['hp']}",
                ):
                    u = kq_units_by[key]
                    if not u.get("emitted"):
                        a = max(a, u["avail"])
                return a

            def emit_dense_for(bi_limit, kind):
                # force all units of `kind` with deadline <= bi_limit
                i = 0
                while i < len(dense):
                    dl, k, u = dense[i]
                    if dl > bi_limit:
                        break
                    if k == kind:
                        dense.pop(i)
                        run_unit(u)
                    else:
                        i += 1

            def emit_dense_one():
                # paced pop: first unit whose inputs have landed
                for i, (dl, k, u) in enumerate(dense):
                    if u.get("avail", 0.0) <= st8["pe"] + 200.0:
                        dense.pop(i)
                        run_unit(u)
                        return True
                return False

            def emit_score(bi):
                b = blocks[bi]
                emit_dense_for(bi, "kq")
                g, hp, j, cs, w0 = b["g"], b["hp"], b["j"], b["cs"], b["w0"]
                # wait for the kq copies this strip depends on
                kd = max(
                    kq_units_by[f"kq{g}f{2 + hp}"]["vend"],
                    kq_units_by[f"kq{j // 4}f{hp}"]["vend"],
                )
                s_t = sp.tile([128, 2 * TQW], f32, tag="s", name="s")
                if bi >= 2:
                    st8["pe"] = max(st8["pe"], exp_end[bi - 2])  # slot free
                st8["pe"] = max(st8["pe"], kd)
                for idx in range(2):
                    kT = kq_s[hp][64 * idx : 64 * idx + 64, :]
                    qT = kq_s[2 + hp][64 * idx : 64 * idx + 64, :]
                    nc.tensor.matmul(
                        s_t[:, 512 * idx + cs : 512 * idx + 512],
                        kT[:, 128 * j : 128 * (j + 1)],
                        qT[:, w0 + cs : w0 + TQW],
                        start=True,
                        stop=True,
                    )
                st8["pe"] += (512 - cs) * RATE + 50
                pt = ptp.tile([128, 2 * TQW], bf16, tag="pt", name="pt")
                nc.scalar.activation(
                    pt[:, cs : 2 * TQW],
                    s_t[:, cs : 2 * TQW],
                    mybir.ActivationFunctionType.Exp,
                    scale=float(D) ** -0.5,
                )
                st8["sc"] = max(st8["sc"], st8["pe"]) + EXPC(2 * (512 - cs))
                exp_end[bi] = st8["sc"]
                if b["diag"]:
                    view = pt.rearrange("p (i x) -> p i x", i=2)[:, :, cs : cs + 128]
                    nc.vector.tensor_tensor(
                        view,
                        view,
                        msk_s.rearrange("p (i x) -> p i x", i=2)[:, :, :],
                        mybir.AluOpType.mult,
                    )
                    st8["vec"] = max(st8["vec"], exp_end[bi]) + VC_MASK
                    mask_end[bi] = st8["vec"]
                else:
                    mask_end[bi] = exp_end[bi]
                strip_pt[bi] = pt

            def emit_pv(bi):
                b = blocks[bi]
                g, hp, j, cs, jmax = b["g"], b["hp"], b["j"], b["cs"], b["jmax"]
                emit_dense_for(bi, "v")
                if j == 0:
                    o_t[0] = [
                        op.tile([128, TQW], f32, tag="oh0", name="oh0"),
                        op.tile([128, TQW], f32, tag="oh1", name="oh1"),
                    ]
                    st8["pe"] = max(st8["pe"], norm_end[0])
                pt = strip_pt[bi]
                for idx in range(2):
                    h = 2 * hp + idx
                    nc.tensor.matmul(
                        o_t[0][idx][:, cs:TQW],
                        v_s[:, 512 * j + 128 * h : 512 * j + 128 * (h + 1)],
                        pt[:, 512 * idx + cs : 512 * idx + 512],
                        start=(j == 0),
                        stop=(j == jmax),
                    )
                t_ready = mask_end[bi]
                if t_ready > st8["pe"]:
                    st8["stall"] += t_ready - st8["pe"]
                    st8["pe"] = t_ready
                st8["pe"] += 2 * (512 - cs) * RATE + 70
                if j == jmax:
                    # normalize this head pair: l rows -> SBUF (one copy on
                    # ScalarE, one on VectorE; both on ScalarE for the very
                    # last one since exp work is done by then), reciprocal,
                    # multiply into oc
                    last = g == NGRP - 1 and hp == 1
                    lsb = rp.tile([128, 512], f32, tag="lsb", name="lsb")
                    rinv = rp.tile([128, 512], f32, tag="rinv", name="rinv")
                    if last or g <= 1:
                        # ScalarE has slack here (or is done); keep VectorE free
                        nc.scalar.copy(lsb[0:64, :], o_t[0][0][64:128, :])
                        st8["sc"] = max(st8["sc"], st8["pe"]) + 700.0
                        if last:
                            nc.scalar.copy(lsb[64:128, :], o_t[0][1][64:128, :])
                            st8["sc"] += 700.0
                            lend = st8["sc"]
                        else:
                            nc.vector.tensor_copy(lsb[64:128, :], o_t[0][1][64:128, :])
                            st8["vec"] = max(st8["vec"], st8["pe"]) + VC_KQ
                            lend = max(st8["sc"], st8["vec"])
                    else:
                        # late windows: ScalarE is the exp bottleneck - use VectorE
                        for idx in range(2):
                            nc.vector.tensor_copy(
                                lsb[64 * idx : 64 * idx + 64, :],
                                o_t[0][idx][64:128, :],
                            )
                        st8["vec"] = max(st8["vec"], st8["pe"]) + 2 * VC_KQ
                        lend = st8["vec"]
                    nc.vector.reciprocal_approx_fast(rinv[:], lsb[:])
                    st8["vec"] = max(st8["vec"], lend) + VC_RECIP
                    for idx in range(2):
                        nc.vector.tensor_tensor(
                            oc_s[hp][64 * idx : 64 * idx + 64, b["w0"] : b["w0"] + TQW],
                            o_t[0][idx][0:64, :],
                            rinv[64 * idx : 64 * idx + 64, :],
                            mybir.AluOpType.mult,
                        )
                        st8["vec"] += VC_MULT
                    norm_end[0] = st8["vec"]
                    if hp == 1:
                        wnorm_end[g] = st8["vec"]
                        for u in proj_units(g):
                            dense.append([NB + 1, "proj", u])

            # greedy interleave ------------------------------------------
            score_i = 0
            pv_i = 0
            LEAD = 2400.0
            while score_i < NB or pv_i < NB:
                can_score = (
                    score_i < NB
                    and (score_i - pv_i) < RING_K
                    and score_avail(score_i) <= st8["pe"] + 300.0
                )
                # defer a score only when the PE would stall on the s_t slot
                # AND ScalarE still has a healthy queue (PE-bound regime)
                score_would_stall = can_score and score_i >= 2 and (
                    exp_end[score_i - 2] > st8["pe"] + 60
                    and st8["sc"] > st8["pe"] + 800.0
                )
                if (
                    can_score
                    and st8["sc"] <= st8["pe"] + LEAD
                    and not score_would_stall
                ):
                    emit_score(score_i)
                    score_i += 1
                    continue
                if pv_i < NB and pv_i < score_i and (
                    mask_end[pv_i] <= st8["pe"] + 60
                ):
                    emit_pv(pv_i)
                    pv_i += 1
                    continue
                if dense and emit_dense_one():
                    continue
                # nothing eligible: during the DMA-bound start, keep the PE
                # clock warm with throwaway matmuls instead of idling
                if st8["pe"] < 32000.0 and ndummy[0] < 100:
                    emit_dummy()
                    continue
                if can_score and st8["sc"] <= st8["pe"] + LEAD:
                    emit_score(score_i)
                    score_i += 1
                elif pv_i < NB and pv_i < score_i:
                    emit_pv(pv_i)
                    pv_i += 1
                elif score_i < NB and (score_i - pv_i) < RING_K:
                    emit_score(score_i)
                    score_i += 1
                else:
                    emit_pv(pv_i)
                    pv_i += 1
            while dense:
                dl, k, u = dense.pop(0)
                run_unit(u)

    nc.compile()
    return nc


def get_nc():
    if "nc" not in _nc_cache:
        _nc_cache["nc"] = _build_bass()
    return _nc_cache["nc"]


def _shard_inputs(x, W_kqv, W_proj):
    """Build the 8 per-core input maps: shard, transpose, cast to bf16 and
    pack 128-row panels along the free dim."""
    bf16 = ml_dtypes.bfloat16

    def pack(a):  # [128*k, n] -> [128, k*n], panel-major along free dim
        k = a.shape[0] // 128
        return np.ascontiguousarray(
            a.reshape(k, 128, a.shape[1]).transpose(1, 0, 2).reshape(128, -1)
        ).astype(bf16)

    in_maps = []
    for core in range(8):
        b, hg = core // 4, core % 4
        heads = range(4 * hg, 4 * hg + 4)
        xt = x[b].T  # [C, T]
        # xt packed per window: [128, g*4096 + c*512 + t']
        xtp = np.ascontiguousarray(
            xt.reshape(8, 128, 4, 512).transpose(1, 2, 0, 3).reshape(128, -1)
        ).astype(bf16)
        k_rows = [W_kqv[64 * h : 64 * (h + 1)] for h in heads]
        q_rows = [W_kqv[C + 64 * h : C + 64 * (h + 1)] for h in heads]
        v_rows = [W_kqv[2 * C + 64 * h : 2 * C + 64 * (h + 1)] for h in heads]
        wqk_cat = np.concatenate(k_rows + q_rows, 0)  # [512 feat, 1024 c]
        # f-major packing: [p, f*1024 + c*128 + fi]
        wqk = np.ascontiguousarray(
            wqk_cat.reshape(4, 128, 8, 128).transpose(3, 0, 2, 1).reshape(128, -1)
        ).astype(bf16)
        wv = pack(np.concatenate(v_rows, 0).T)
        wp = pack(W_proj[:, 256 * hg : 256 * (hg + 1)].T)
        in_maps.append({"xt": xtp, "wqk": wqk, "wv": wv, "wp": wp})
    return in_maps


def kernel(x, W_kqv, W_proj, b_proj):
    from concourse.bass_utils import run_bass_kernel_spmd

    x = np.asarray(x, dtype=np.float32)
    W_kqv = np.asarray(W_kqv, dtype=np.float32)
    W_proj = np.asarray(W_proj, dtype=np.float32)
    b_proj = np.asarray(b_proj, dtype=np.float32)
    nc = get_nc()
    in_maps = _shard_inputs(x, W_kqv, W_proj)
    res = run_bass_kernel_spmd(nc, in_maps, core_ids=list(range(8)))
    B = x.shape[0]
    out = np.empty((B, T, C), np.float32)
    for b in range(B):
        acc = res.results[4 * b]["y"].astype(np.float32).copy()
        for hg in range(1, 4):
            acc += res.results[4 * b + hg]["y"]
        out[b] = acc + b_proj[None, :]
    return out
